# revision 1
# baseline (speedup 1.0000x reference)
"""Trainium2 Bass kernel for nn_DCWTv2InferenceCache (segment-tree cached attention).

Sharding: tensor-parallel over the 16-head axis -> 8 cores x 2 heads.
Each core streams its (50000, 2*64) f32 slice of the value cache from HBM,
reduces segment-tree nodes to (64, 128) block-sums on the PE (selection-matrix
matmul accumulating in PSUM), then runs the per-node depth-projected attention
epilogue fully on-device. Output is head-sharded (2, 64) per core, gathered on
host. No cross-device communication.
"""

import math
import os
import sys

if "/opt/trn_rl_repo" not in sys.path:
    sys.path.insert(0, "/opt/trn_rl_repo")

import numpy as np

import concourse.bass as bass
import concourse.mybir as mybir
import concourse.tile as tile
from concourse import bacc
from concourse.bass_utils import run_bass_kernel_spmd

# --- problem constants (from the reference nn.Module) ---
MAX_LEN = 65536
NUM_HEADS = 16
HEAD_DIM = 64
K_MAX = 64
LOCAL_WINDOW = 512
LOG_N = 17
LEAF_START = 2**LOG_N

N_CORES = 8
HPC = NUM_HEADS // N_CORES        # heads per core = 2
F = HPC * HEAD_DIM                # feature width per core = 128
NTOK = 50000                      # v_tokens buffer length

CHUNK = 128                       # tokens per matmul tile (partition dim)
SUP = 4                           # chunks per DMA (old path, small nodes)

# Stage-A (streaming block-sum) arithmetic mode: "f32" | "bf16" | "f32r"
STAGE_A_MODE = os.environ.get("DCWT_STAGE_A_MODE", "r64")
SEL64_BCAST = os.environ.get("DCWT_SEL64_BCAST", "0") == "1"


def _cblob_layout(NT):
    """Column offsets inside the packed (128, W) f32 constants blob."""
    nt = max(NT, 1)
    off = {}
    off["ident"] = 0
    off["qbd"] = 128
    off["qT"] = 130
    off["temps"] = 132
    off["wTI"] = 132 + nt
    return off, 132 + nt + nt * 64

f32 = mybir.dt.float32
AF = mybir.ActivationFunctionType
AX = mybir.AxisListType

_last_results = None  # stash for test harness introspection


def cover_set(pos):
    """O(log n) segment-tree nodes covering prefix [0..pos-1]: (start, L, depth),
    ascending start / descending L (binary decomposition of pos)."""
    if pos <= 0:
        return []
    l, r = LEAF_START, LEAF_START + min(pos, MAX_LEN)
    out = []
    while l < r:
        if l & 1:
            d = LOG_N - int(math.floor(math.log2(l)))
            out.append(((l << d) - LEAF_START, 1 << d, d))
            l += 1
        if r & 1:
            r -= 1
            d = LOG_N - int(math.floor(math.log2(r)))
            out.append(((r << d) - LEAF_START, 1 << d, d))
        l >>= 1
        r >>= 1
    return sorted(out)


def _build_program(pos, mode):
    """Build the single-core Bass/Tile program (same program for all 8 cores)."""
    nodes = cover_set(pos)
    big = [(s, L, d) for (s, L, d) in nodes if L > K_MAX]      # L >= 128, 128-aligned
    small = [(s, L, d) for (s, L, d) in nodes if L <= K_MAX]   # raw tail nodes
    tree = big + small                                          # epilogue order
    NT = len(tree)
    n_loc = min(pos, LOCAL_WINDOW)
    assert n_loc % CHUNK == 0, "local window must be chunk-aligned for this build"
    NLC = n_loc // CHUNK

    inv_sqrt_d = 1.0 / math.sqrt(HEAD_DIM)

    if mode == "bf16":
        mm_dt = mybir.dt.bfloat16
    elif mode in ("f32r", "r64"):
        mm_dt = mybir.dt.float32r
    else:
        mm_dt = f32

    nc = bacc.Bacc("TRN2", target_bir_lowering=False, debug=False)

    v = nc.dram_tensor("v", [NTOK, F], f32, kind="ExternalInput")
    SELB_W = K_MAX * K_MAX + K_MAX if mode == "r64" else K_MAX
    selb_d = nc.dram_tensor("selb", [CHUNK, SELB_W], mm_dt, kind="ExternalInput")
    CBOFF, CB_W = _cblob_layout(NT)
    CB_IDENT, CB_QBD, CB_QT = CBOFF["ident"], CBOFF["qbd"], CBOFF["qT"]
    CB_TEMPS, CB_WTI = CBOFF["temps"], CBOFF["wTI"]
    cblob_d = nc.dram_tensor("cblob", [CHUNK, CB_W], f32, kind="ExternalInput")
    o = nc.dram_tensor("o", [HPC, HEAD_DIM], f32, kind="ExternalOutput")

    with tile.TileContext(nc) as tc:
        with (
            tc.tile_pool(name="consts", bufs=1) as cpool,
            tc.tile_pool(name="vstream", bufs=4) as vpool,
            tc.tile_pool(name="fsb", bufs=2) as fpool,
            tc.tile_pool(name="ep_sb", bufs=2) as spool,
            tc.tile_pool(name="xsb", bufs=3) as xpool,
            tc.tile_pool(name="acc_ps", bufs=1, space=bass.MemorySpace.PSUM) as apool,
            tc.tile_pool(name="ep_ps", bufs=1, space=bass.MemorySpace.PSUM) as eppool,
            tc.tile_pool(name="out_ps", bufs=1, space=bass.MemorySpace.PSUM) as opool,
        ):
            # ---- constants: one blob DMA for all small consts (the HW
            # allows only ~10 outstanding DMA queues; fewer issues = no
            # head-of-line stalls). Scalar HWDGE ring; sync ring carries only
            # the big token stream. gpsimd (SWDGE) starves behind the stream
            # and is never used.
            cb = cpool.tile([CHUNK, CB_W], f32)
            nc.sync.dma_start(cb[:], cblob_d[:])
            ident_sb = cb[:, CB_IDENT : CB_IDENT + CHUNK]
            qbd_sb = cb[:, CB_QBD : CB_QBD + HPC]
            qT_sb = cb[0:HEAD_DIM, CB_QT : CB_QT + HPC]
            temps2_sb = cb[0:HPC, CB_TEMPS : CB_TEMPS + max(NT, 1)]

            selb = cpool.tile([CHUNK, SELB_W], mm_dt)
            sel_sb = selb[:, SELB_W - K_MAX : SELB_W]

            def wTI_slice(n):
                return cb[0:HEAD_DIM, CB_WTI + n * HEAD_DIM : CB_WTI + (n + 1) * HEAD_DIM]

            # ---- prefetch raw tail data (small nodes + local window) ----
            small_tiles = []
            for si, (start_s, L_s, _d) in enumerate(small):
                fsm = cpool.tile([K_MAX, F], f32, name=f"fsm{si}", tag=f"fsm{si}")
                nc.sync.dma_start(fsm[0:L_s, :], v[start_s : start_s + L_s, :])
                small_tiles.append(fsm)
            lstart = pos - n_loc
            fl_sb = cpool.tile([CHUNK, NLC, F], f32)
            nc.sync.dma_start(
                fl_sb[:],
                v[lstart : lstart + n_loc, :].rearrange("(c p) f -> p c f", p=CHUNK),
            )

            def sel64_lhsT(r):
                return selb[:, r * K_MAX : (r + 1) * K_MAX]

            # ---- per-node softmax scales: 1/((softplus(t)+1e-6)*sqrt(D)) ----
            et_sb = cpool.tile([HPC, max(NT, 1)], f32)
            nc.scalar.activation(et_sb[:], temps2_sb, AF.Exp)
            sp_sb = cpool.tile([HPC, max(NT, 1)], f32)
            nc.scalar.activation(sp_sb[:], et_sb[:], AF.Ln, bias=1.0)  # softplus
            u_sb = cpool.tile([HPC, max(NT, 1)], f32)
            # u = (sp + 1e-6) * sqrt(D) = sp*sqrt(D) + 1e-6*sqrt(D)
            nc.scalar.mul(u_sb[:], sp_sb, math.sqrt(HEAD_DIM))
            nc.vector.tensor_scalar_add(u_sb[:], u_sb[:], 1e-6 * math.sqrt(HEAD_DIM))
            rs_sb = cpool.tile([HPC, max(NT, 1)], f32)
            nc.vector.reciprocal(rs_sb[:], u_sb[:])
            ns_sb = cpool.tile([HPC, max(NT, 1)], f32)
            nc.scalar.mul(ns_sb[:], rs_sb[:], -1.0)

            # wave-2 prefetch: small big-nodes (old-path layout), per chunk;
            # issued late so the outstanding-DMA-queue count stays < 10
            oldpath_tiles = {}
            for (start_b, L_b, _d) in big:
                if mode == "r64" and L_b >= CHUNK * K_MAX:
                    continue
                nch_b = L_b // CHUNK
                vo = cpool.tile(
                    [CHUNK, nch_b, F], mm_dt, name=f"vo{start_b}", tag=f"vo{start_b}"
                )
                for cb_i in range(nch_b):
                    srcb = v[start_b + cb_i * CHUNK : start_b + (cb_i + 1) * CHUNK, :]
                    if mode in ("f32r", "r64"):
                        srcb = srcb.bitcast(mybir.dt.float32r)
                    nc.sync.dma_start(vo[:, cb_i, :], srcb)
                oldpath_tiles[start_b] = vo
            # sel matrices last among the prefetches, right before the stream
            nc.sync.dma_start(selb[:], selb_d[:])

            # ---- all tree-node q_depth projections upfront (block-diag) ----
            qd_all = cpool.tile([2 * HEAD_DIM, max(NT, 1), HPC], f32)
            nc.vector.memset(qd_all[:], 0.0)
            for n in range(NT):
                qd_ps = eppool.tile([2 * HEAD_DIM, HPC], f32, tag="qd_ps")
                nc.tensor.matmul(
                    qd_ps[0:HEAD_DIM, 0:1],
                    wTI_slice(n), qT_sb[:, 0:1], start=True, stop=True,
                )
                nc.tensor.matmul(
                    qd_ps[HEAD_DIM : 2 * HEAD_DIM, 1:2],
                    wTI_slice(n), qT_sb[:, 1:2], start=True, stop=True,
                )
                nc.scalar.copy(qd_all[0:HEAD_DIM, n, 0:1], qd_ps[0:HEAD_DIM, 0:1])
                nc.scalar.copy(
                    qd_all[HEAD_DIM : 2 * HEAD_DIM, n, 1:2],
                    qd_ps[HEAD_DIM : 2 * HEAD_DIM, 1:2],
                )

            # ---- cross-node output accumulator (2, 128) PSUM ----
            out_ps = opool.tile([HPC, F], f32)
            n_out_mm = len(tree) + NLC
            out_mm = [0]  # running count, for start/stop flags

            def out_matmul(wT_sb_ap, f_sb_ap):
                nc.tensor.matmul(
                    out_ps[:], wT_sb_ap, f_sb_ap,
                    start=(out_mm[0] == 0), stop=(out_mm[0] == n_out_mm - 1),
                )
                out_mm[0] += 1

            def softmax_weights(s_ps_ap, K, node_i, is_tree):
                """softmax over K free-dim entries of (2, K) logits (pre-scale);
                returns (2, K) SBUF weights; tree weights folded by 1/NT."""
                smax = xpool.tile([HPC, 1], f32, tag="smax")
                nc.vector.reduce_max(smax[:], s_ps_ap, axis=AX.X)
                biast = xpool.tile([HPC, 1], f32, tag="biast")
                ebd = xpool.tile([HPC, K], f32, tag="esb")
                zt = xpool.tile([HPC, 1], f32, tag="zt")
                if is_tree:
                    nc.vector.tensor_scalar_mul(
                        biast[:], smax[:], ns_sb[:, node_i : node_i + 1]
                    )
                    nc.scalar.activation(
                        ebd[:], s_ps_ap, AF.Exp,
                        bias=biast[:], scale=rs_sb[:, node_i : node_i + 1],
                        accum_out=zt[:],
                    )
                else:
                    nc.scalar.mul(biast[:], smax[:], -inv_sqrt_d)
                    nc.scalar.activation(
                        ebd[:], s_ps_ap, AF.Exp, bias=biast[:], scale=inv_sqrt_d,
                        accum_out=zt[:],
                    )
                if is_tree:
                    zs = xpool.tile([HPC, 1], f32, tag="zs")
                    nc.scalar.mul(zs[:], zt[:], float(NT))
                    zt = zs
                rz = xpool.tile([HPC, 1], f32, tag="rz")
                nc.vector.reciprocal(rz[:], zt[:])
                w_sb = xpool.tile([HPC, K], f32, tag="wsb")
                nc.vector.tensor_scalar_mul(w_sb[:], ebd[:], rz[:])
                return w_sb

            def tree_epilogue(node_i, f_sb_ap, K):
                """Attention of depth-projected query against f (K, F) for one node."""
                # fT (F, K) for the logits matmul
                fT_ps = eppool.tile([F, K_MAX], f32, tag="fT_ps", bufs=2)
                nc.tensor.transpose(fT_ps[:, 0:K], f_sb_ap, ident_sb[0:K, 0:K])
                fT_sb = spool.tile([F, K_MAX], f32, tag="fT_sb")
                nc.scalar.copy(fT_sb[:, 0:K], fT_ps[:, 0:K])
                # logits (2, K), pre-scale; qd_all holds block-diag q_depth
                s_ps = eppool.tile([HPC, K_MAX], f32, tag="s_ps", bufs=2)
                nc.tensor.matmul(
                    s_ps[:, 0:K], qd_all[:, node_i, :], fT_sb[:, 0:K],
                    start=True, stop=True,
                )
                w_sb = softmax_weights(s_ps[:, 0:K], K, node_i, True)
                # out += wT.T @ f  (folded by 1/NT via Z scaling)
                wT_ps = eppool.tile([K_MAX, HPC], f32, tag="wT_ps")
                nc.tensor.transpose(wT_ps[0:K, :], w_sb[:], ident_sb[0:HPC, 0:HPC])
                wT_sb = spool.tile([K_MAX, HPC], f32, tag="wT_sb")
                nc.scalar.copy(wT_sb[0:K, :], wT_ps[0:K, :])
                out_matmul(wT_sb[0:K, :], f_sb_ap)

            # ================= emission schedule =================
            # PE is strict FIFO, so order matters: first r64 node streams
            # while its MMs chase the DMA; the prefetched small/old/local
            # epilogues fill the PE+ACT gap while the LAST r64 node's data is
            # still in flight; that node's epilogue is the only tail.
            BLK = CHUNK * K_MAX  # 8192 tokens per linear c-chunk (r64 path)

            def emit_r64_node(node_i, start, L):
                CC = L // BLK
                ps2 = apool.tile([K_MAX, 2, F], f32, tag="acc")
                n_mm = ((CC + 1) // 2) * K_MAX
                done = 0
                for c0 in range(0, CC, 2):
                    ncc = min(2, CC - c0)
                    vt = vpool.tile(
                        [CHUNK, 2, BLK // CHUNK * F], mm_dt, tag="vbig", bufs=2
                    )
                    src = v[start + c0 * BLK : start + (c0 + ncc) * BLK, :]
                    src = src.bitcast(mybir.dt.float32r)
                    src = src.rearrange("(c q r) f -> q c (r f)", q=CHUNK, r=K_MAX)
                    nc.sync.dma_start(vt[:, 0:ncc, :], src)
                    for r in range(K_MAX):
                        nc.tensor.matmul(
                            ps2[:, 0:ncc, :], sel64_lhsT(r),
                            vt[:, 0:ncc, r * F : (r + 1) * F],
                            start=(done == 0), stop=(done == n_mm - 1),
                        )
                        done += 1
                f_sb = fpool.tile([K_MAX, F], f32, tag="f")
                mean_scale = float(K_MAX) / L
                if CC > 1:
                    ha = fpool.tile([K_MAX, F], f32, tag="ha")
                    nc.scalar.mul(ha[:], ps2[:, 0, :], mean_scale)
                    hb = fpool.tile([K_MAX, F], f32, tag="hb")
                    nc.scalar.mul(hb[:], ps2[:, 1, :], mean_scale)
                    nc.vector.tensor_add(f_sb[:], ha[:], hb[:])
                else:
                    nc.scalar.mul(f_sb[:], ps2[:, 0, :], mean_scale)
                tree_epilogue(node_i, f_sb[:], K_MAX)

            def emit_old_node(node_i, start, L):
                nch = L // CHUNK
                vt = oldpath_tiles[start]
                if mode in ("f32r", "r64"):
                    ps2 = apool.tile([K_MAX, 2, F], f32, tag="acc")
                else:
                    ps = apool.tile([K_MAX, F], f32, tag="acc")
                done = 0
                if mode in ("f32r", "r64"):
                    c = 0
                    while c < nch:
                        w = 2 if c + 2 <= nch else 1
                        nc.tensor.matmul(
                            ps2[:, 0:w, :], sel_sb[:], vt[:, c : c + w, :],
                            start=(done == 0), stop=(done + w == nch),
                        )
                        done += w
                        c += w
                else:
                    for c in range(nch):
                        nc.tensor.matmul(
                            ps[:], sel_sb[:], vt[:, c, :],
                            start=(done == 0), stop=(done == nch - 1),
                        )
                        done += 1
                f_sb = fpool.tile([K_MAX, F], f32, tag="f")
                mean_scale = float(K_MAX) / L
                if mode in ("f32r", "r64"):
                    if nch > 1:
                        ha = fpool.tile([K_MAX, F], f32, tag="ha")
                        nc.scalar.mul(ha[:], ps2[:, 0, :], mean_scale)
                        hb = fpool.tile([K_MAX, F], f32, tag="hb")
                        nc.scalar.mul(hb[:], ps2[:, 1, :], mean_scale)
                        nc.vector.tensor_add(f_sb[:], ha[:], hb[:])
                    else:
                        nc.scalar.mul(f_sb[:], ps2[:, 0, :], mean_scale)
                else:
                    nc.scalar.mul(f_sb[:], ps[:], mean_scale)
                tree_epilogue(node_i, f_sb[:], K_MAX)

            def emit_smalls():
                for si, (start, L, depth) in enumerate(small):
                    tree_epilogue(len(big) + si, small_tiles[si][0:L, :], L)

            def emit_local():
                fTl_ps = eppool.tile([F, NLC * CHUNK], f32, tag="fT_ps", bufs=2)
                for c in range(NLC):
                    nc.tensor.transpose(
                        fTl_ps[:, c * CHUNK : (c + 1) * CHUNK], fl_sb[:, c, :],
                        ident_sb[:],
                    )
                fTl_sb = spool.tile([F, NLC * CHUNK], f32, tag="fTl_sb")
                nc.scalar.copy(fTl_sb[:], fTl_ps[:])
                sl_ps = eppool.tile([HPC, NLC * CHUNK], f32, tag="s_ps", bufs=2)
                nc.tensor.matmul(sl_ps[:], qbd_sb, fTl_sb[:], start=True, stop=True)
                wl_sb = softmax_weights(sl_ps[:], n_loc, -1, False)
                for c in range(NLC):
                    wTl_ps = eppool.tile([CHUNK, HPC], f32, tag="wT_ps")
                    nc.tensor.transpose(
                        wTl_ps[:], wl_sb[:, c * CHUNK : (c + 1) * CHUNK],
                        ident_sb[0:HPC, 0:HPC],
                    )
                    wTl_sb = spool.tile([CHUNK, HPC], f32, tag="wTl_sb")
                    nc.scalar.copy(wTl_sb[:], wTl_ps[:])
                    out_matmul(wTl_sb[:], fl_sb[:, c, :])

            is_r64 = [mode == "r64" and L >= BLK for (_s, L, _d) in big]
            r64_idx = [i for i in range(len(big)) if is_r64[i]]
            old_idx = [i for i in range(len(big)) if not is_r64[i]]
            emit_local()
            emit_smalls()
            for i in old_idx:
                emit_old_node(i, big[i][0], big[i][1])
            for i in r64_idx:
                emit_r64_node(i, big[i][0], big[i][1])

            # ================= final output =================
            acc_sb = spool.tile([HPC, F], f32, tag="acc_sb")
            nc.scalar.copy(acc_sb[:], out_ps[:])
            # head h's output lives at acc_sb[h, h*64:(h+1)*64]; DMA handles the
            # partition-base-1 read that compute engines can't.
            nc.sync.dma_start(o[0:1, :], acc_sb[0:1, 0:HEAD_DIM])
            nc.sync.dma_start(o[1:2, :], acc_sb[1:2, HEAD_DIM : 2 * HEAD_DIM])

    nc.compile()
    return nc


def _make_in_maps(v_tokens, q_new, depth_proj_w, depth_temp, pos):
    nodes = cover_set(pos)
    big = [(st, L, d) for (st, L, d) in nodes if L > K_MAX]
    small = [(st, L, d) for (st, L, d) in nodes if L <= K_MAX]
    tree = big + small
    NT = len(tree)
    OFF, CB_W = _cblob_layout(NT)

    sel = np.tile(np.eye(K_MAX, dtype=np.float32), (CHUNK // K_MAX, 1))
    if STAGE_A_MODE == "r64":
        sel64 = np.broadcast_to(
            np.eye(K_MAX, dtype=np.float32), (CHUNK, K_MAX, K_MAX)
        ).reshape(CHUNK, K_MAX * K_MAX)
        selb = np.concatenate([sel64, sel], axis=1)
    else:
        selb = sel

    wTI = np.stack(
        [np.eye(HEAD_DIM, dtype=np.float32) + depth_proj_w[d].T for (_, _, d) in tree]
    ) if NT else np.zeros((1, HEAD_DIM, HEAD_DIM), np.float32)
    tsel = np.array([depth_temp[d] for (_, _, d) in tree], np.float32) \
        if NT else np.zeros((1,), np.float32)

    in_maps = []
    for c in range(N_CORES):
        q_c = q_new[0, HPC * c : HPC * (c + 1), :]          # (2, 64)
        cb = np.zeros((CHUNK, CB_W), np.float32)
        cb[:, OFF["ident"] : OFF["ident"] + CHUNK] = np.eye(CHUNK)
        for h in range(HPC):
            cb[h * HEAD_DIM : (h + 1) * HEAD_DIM, OFF["qbd"] + h] = q_c[h]
        cb[0:HEAD_DIM, OFF["qT"] : OFF["qT"] + HPC] = q_c.T
        cb[0:HPC, OFF["temps"] : OFF["temps"] + max(NT, 1)] = tsel[None, :]
        for n in range(max(NT, 1)):
            cb[0:HEAD_DIM, OFF["wTI"] + n * HEAD_DIM : OFF["wTI"] + (n + 1) * HEAD_DIM] = (
                wTI[n] if NT else 0.0
            )
        im = {
            "v": np.ascontiguousarray(
                v_tokens[:, HPC * c : HPC * (c + 1), :]
            ).reshape(NTOK, F),
            "selb": np.ascontiguousarray(selb),
            "cblob": cb,
        }
        in_maps.append(im)
    return in_maps


def kernel(v_tokens, q_new, depth_proj_w, depth_temp, n_tokens, _profile=False):
    global _last_results
    v_tokens = np.asarray(v_tokens, dtype=np.float32)
    q_new = np.asarray(q_new, dtype=np.float32)
    depth_proj_w = np.asarray(depth_proj_w, dtype=np.float32)
    depth_temp = np.asarray(depth_temp, dtype=np.float32)
    pos = int(n_tokens)

    nc = _build_program(pos, STAGE_A_MODE)
    in_maps = _make_in_maps(v_tokens, q_new, depth_proj_w, depth_temp, pos)
    res = run_bass_kernel_spmd(
        nc, in_maps, core_ids=list(range(N_CORES)), trace=_profile
    )
    _last_results = res

    out = np.zeros((1, NUM_HEADS, HEAD_DIM), np.float32)
    for c in range(N_CORES):
        out[0, HPC * c : HPC * (c + 1), :] = res.results[c]["o"]
    return out

